# revision 33
# baseline (speedup 1.0000x reference)
"""Trainium2 Bass kernel for causal linear-attention approximation.

Reference computation (per batch b, head h):
  q,k = hidden @ Wq|Wk -> (L, F=16);  v = hidden @ Wv -> (L, DH=64)
  ck = k - cummean(k);  cv = v - cummean(v)        (cumsums over seq)
  qK[i,j] = q_i . ck_j   (causal: j<=i)
  s[i] = sum_j qK[i,j]^2 / (2*DH);  qKsq = cumsum_i(s);  den = (i+1)+qKsq
  y = cummean(v) + (qK @ cv) / (sqrt(DH) * den)
  out = concat_heads(y) @ Wo

Distribution: 8 cores = 2 batches x 4 head-groups (3 heads each). Each core
computes a partial (L, D) output = y_heads @ Wo_rows; host sums 4 partials
per batch.

Causal-block decomposition: for query chunk jq (512 queries), keys in earlier
chunks contribute only through running 16x16 covariance C = ck^T ck and 16x64
cross G = ck^T cv; only the 4 diagonal key blocks need explicit qK tiles.

Layout note: every per-head partition offset is 32-aligned (head h lives at
partitions [32h, 32h+16) with 16 pad rows) — walrus rejects non-32-aligned
partition bases on compute-engine APs.

Phase 1 (prefix) is emitted chunk-major so projections (PE), scans/centering
(DVE), psum copies (ACT/DVE) and transposes pipeline against each other.
"""

import numpy as np

import concourse.bacc as bacc
import concourse.mybir as mybir
import concourse.tile as tile
from concourse.masks import make_identity

F32 = mybir.dt.float32
F32R = mybir.dt.float32r
ADD = mybir.AluOpType.add
BYPASS = mybir.AluOpType.bypass

B, L, D = 2, 2048, 768
H, F, DH = 12, 16, 64
HPC = 3                 # heads per core
NCORES = 8
NB = L // 128           # 16 key blocks
NQ = L // 512           # 4 query chunks
QC = 512                # query chunk size
KB = 128                # key block size
PH = 96                 # padded per-head partition span (3 heads x 32)
INV2DH8 = 8.0 / (2.0 * DH)   # s-reduce weight: folds 1/(2*DH) and the x8

# epilogue recip broadcast: "dve" = stride-0 partition AP on DVE (fast path),
# "gpsimd" = partition_broadcast via a staged base-0 row (validated fallback)
BCAST_MODE = "gpsimd"


def build_nc():
    nc = bacc.Bacc("TRN2", target_bir_lowering=False, debug=False)

    hT = nc.declare_dram_parameter("hT", [D, L], F32, isOutput=False)
    # wq/wk padded: head h at columns [32h, 32h+16), zeros elsewhere
    wq = nc.declare_dram_parameter("wq", [D, PH], F32, isOutput=False)
    wk = nc.declare_dram_parameter("wk", [D, PH], F32, isOutput=False)
    wv = nc.declare_dram_parameter("wv", [D, HPC * DH], F32, isOutput=False)
    wo = nc.declare_dram_parameter("wo", [HPC * DH, D], F32, isOutput=False)
    nv8 = nc.declare_dram_parameter("nv8", [PH, L], F32, isOutput=False)
    invn = nc.declare_dram_parameter("invn", [128, L], F32, isOutput=False)
    cst = nc.declare_dram_parameter("cst", [128, 1], F32, isOutput=False)
    out_e = nc.declare_dram_parameter("out", [L, D], F32, isOutput=True)

    with tile.TileContext(nc) as tc:
        with (
            tc.tile_pool(name="const", bufs=1) as cpool,
            tc.tile_pool(name="wout", bufs=1) as wopool,
            tc.tile_pool(name="big", bufs=1) as bpool,
        ):
            # ---------- persistent big tiles ----------
            qt_sb = bpool.tile([PH, L], F32R, tag="qt_sb")
            kt_sb = bpool.tile([PH, L], F32R, tag="kt_sb")
            kscan = bpool.tile([PH, L], F32, tag="kscan")      # raw k cumsum
            vt_hi = bpool.tile([128, L], F32, tag="vt_hi")     # vT -> cvT
            vt_lo = bpool.tile([64, L], F32, tag="vt_lo")
            vs_hi = bpool.tile([128, L], F32, tag="vs_hi")     # raw v cumsum
            vs_lo = bpool.tile([64, L], F32, tag="vs_lo")
            mv_hi = bpool.tile([128, L], F32, tag="mv_hi")     # mean_vT
            mv_lo = bpool.tile([64, L], F32, tag="mv_lo")
            cv_nat = bpool.tile([128, NB, HPC * DH], F32R, tag="cv_nat")
            ck_nat = bpool.tile([128, NB, PH], F32R, tag="ck_nat")
            cg_run = bpool.tile([PH, DH + F], F32, tag="cg_run")
            cg_sb = bpool.tile([PH, 3, DH + F], F32R, tag="cg_sb")
            yt_lo = bpool.tile([64, L], F32R, tag="yt_lo")

            # ---------- phase 1: proj + center + transpose + C/G ----------
            with (
                tc.tile_pool(name="wproj", bufs=1) as wppool,
                tc.tile_pool(name="ht", bufs=7) as htpool,
                tc.tile_pool(name="ktmp", bufs=3) as ktmppool,
                tc.tile_pool(name="pp", bufs=1, space="PSUM") as pp,
                tc.tile_pool(name="ptr", bufs=1, space="PSUM") as ptr,
                tc.tile_pool(name="pcg", bufs=2, space="PSUM") as pcg,
            ):
                wq_sb = wppool.tile([128, 6, PH], F32R)
                nc.sync.dma_start(
                    wq_sb[:],
                    wq[:].rearrange("(c p) f -> p c f", p=128).bitcast(F32R))
                wk_sb = wppool.tile([128, 6, PH], F32R)
                nc.sync.dma_start(
                    wk_sb[:],
                    wk[:].rearrange("(c p) f -> p c f", p=128).bitcast(F32R))
                wv_sb = wppool.tile([128, 6, HPC * DH], F32R)
                nc.sync.dma_start(
                    wv_sb[:],
                    wv[:].rearrange("(c p) f -> p c f", p=128).bitcast(F32R))

                # constants (loaded after the critical-path weight DMAs)
                idt = cpool.tile([128, 128], F32)
                make_identity(nc, idt[:])
                ones_sc = cpool.tile([128, 1], F32R)
                nc.scalar.dma_start(ones_sc[:], cst[:].bitcast(F32R))
                invn_row = cpool.tile([1, L], F32)
                nc.scalar.dma_start(invn_row[:], invn[0:1, :])
                invn_bc = cpool.tile([128, L], F32)
                nc.gpsimd.partition_broadcast(invn_bc[:], invn_row[0:1, :])
                nv8_row = cpool.tile([1, L], F32)
                nc.scalar.dma_start(nv8_row[:], nv8[0:1, :])
                nv8_96 = cpool.tile([PH, L], F32)
                nc.gpsimd.partition_broadcast(nv8_96[:], nv8_row[0:1, :])
                masks = []
                for g in range(4):
                    m = cpool.tile([128, QC], F32, name=f"mask{g}",
                                   tag=f"mask{g}")
                    nc.gpsimd.memset(m[:], 1.0)
                    nc.gpsimd.affine_select(
                        out=m[:], in_=m[:], compare_op=mybir.AluOpType.is_ge,
                        fill=0.0, base=-128 * g, pattern=[[1, QC]],
                        channel_multiplier=-1,
                    )
                    masks.append(m)
                wo_hi = wopool.tile([128, D], F32R)
                wo_lo = wopool.tile([64, D], F32R)
                nc.scalar.dma_start(wo_hi[:], wo[0:128, :].bitcast(F32R))
                nc.scalar.dma_start(wo_lo[:], wo[128:192, :].bitcast(F32R))

                for jq in range(NQ):
                    qs = slice(QC * jq, QC * (jq + 1))
                    # --- projections for this chunk (two psum sub-passes) ---
                    hts = []
                    p_q = pp.tile([PH, QC], F32, name="psq", tag="pq")
                    p_k = pp.tile([PH, QC], F32, name="psk", tag="pk")
                    for k in range(6):
                        ht_t = htpool.tile([128, QC], F32R, name="ht",
                                           tag="ht")
                        hts.append(ht_t)
                        nc.sync.dma_start(
                            ht_t[:],
                            hT[128 * k:128 * (k + 1), qs].bitcast(F32R))
                        st, sp = (k == 0), (k == 5)
                        nc.tensor.matmul(p_q[:], wq_sb[:, k, :], ht_t[:],
                                         start=st, stop=sp)
                        nc.tensor.matmul(p_k[:], wk_sb[:, k, :], ht_t[:],
                                         start=st, stop=sp)
                    nc.scalar.copy(qt_sb[:, qs], p_q[:])
                    nc.scalar.copy(kt_sb[:, qs], p_k[:])
                    p_vh = pp.tile([128, QC], F32, name="psvh", tag="pq")
                    p_vl = pp.tile([64, QC], F32, name="psvl", tag="pk")
                    for k in range(6):
                        st, sp = (k == 0), (k == 5)
                        nc.tensor.matmul(p_vh[:], wv_sb[:, k, 0:128],
                                         hts[k][:], start=st, stop=sp)
                        nc.tensor.matmul(p_vl[:], wv_sb[:, k, 128:192],
                                         hts[k][:], start=st, stop=sp)
                    nc.scalar.copy(vt_hi[:, qs], p_vh[:])
                    nc.scalar.copy(vt_lo[:, qs], p_vl[:])

                    # --- centering for this chunk (chained scans) ---
                    ik = (0.0 if jq == 0 else kscan[:, QC * jq - 1:QC * jq])
                    nc.vector.tensor_tensor_scan(
                        kscan[:, qs], kt_sb[:, qs].bitcast(F32),
                        kt_sb[:, qs].bitcast(F32), ik, ADD, BYPASS)
                    ktmp = ktmppool.tile([PH, QC], F32, name="ktmp",
                                         tag="ktmp")
                    nc.vector.tensor_mul(ktmp[:], kscan[:, qs],
                                         invn_bc[0:PH, qs])
                    nc.vector.tensor_sub(kt_sb[:, qs],
                                         kt_sb[:, qs].bitcast(F32), ktmp[:])
                    ih = (0.0 if jq == 0 else vs_hi[:, QC * jq - 1:QC * jq])
                    nc.vector.tensor_tensor_scan(
                        vs_hi[:, qs], vt_hi[:, qs], vt_hi[:, qs],
                        ih, ADD, BYPASS)
                    il = (0.0 if jq == 0 else vs_lo[:, QC * jq - 1:QC * jq])
                    nc.vector.tensor_tensor_scan(
                        vs_lo[:, qs], vt_lo[:, qs], vt_lo[:, qs],
                        il, ADD, BYPASS)
                    nc.vector.tensor_mul(mv_hi[:, qs], vs_hi[:, qs],
                                         invn_bc[0:128, qs])
                    nc.vector.tensor_mul(mv_lo[:, qs], vs_lo[:, qs],
                                         invn_bc[0:64, qs])
                    nc.vector.tensor_sub(vt_hi[:, qs], vt_hi[:, qs],
                                         mv_hi[:, qs])   # cvT
                    nc.vector.tensor_sub(vt_lo[:, qs], vt_lo[:, qs],
                                         mv_lo[:, qs])   # cvT

                    # --- transposes for this chunk's 4 key blocks ---
                    for lb in range(4 * jq, 4 * (jq + 1)):
                        cs = slice(128 * lb, 128 * (lb + 1))
                        tch = ptr.tile([128, 128], F32, name="tch", tag="tch")
                        nc.tensor.transpose(tch[:], vt_hi[:, cs], idt[:])
                        nc.scalar.copy(cv_nat[:, lb, 0:128], tch[:])
                        tcl = ptr.tile([128, 64], F32, name="tcl", tag="tcl")
                        nc.tensor.transpose(tcl[:], vt_lo[:, cs],
                                            idt[0:64, 0:64])
                        nc.scalar.copy(cv_nat[:, lb, 128:192], tcl[:])
                        tck = ptr.tile([128, PH], F32, name="tck", tag="tck")
                        nc.tensor.transpose(tck[:], kt_sb[:, cs].bitcast(F32),
                                            idt[0:PH, 0:PH])
                        nc.scalar.copy(ck_nat[:, lb, :], tck[:])

                    # --- C/G prefix snapshot (covers blocks of chunk jq-1) ---
                    if jq == 0:
                        nc.vector.memset(cg_run[:], 0.0)
                    else:
                        for h in range(HPC):
                            hs = slice(32 * h, 32 * h + F)
                            dl = pcg.tile([F, DH + F], F32, name="dl",
                                          tag="cgd")
                            for i in range(4):
                                bk = 4 * (jq - 1) + i
                                nc.tensor.matmul(
                                    dl[:, 0:DH],
                                    ck_nat[:, bk, hs],
                                    cv_nat[:, bk, DH * h:DH * (h + 1)],
                                    start=(i == 0), stop=(i == 3))
                                nc.tensor.matmul(
                                    dl[:, DH:DH + F],
                                    ck_nat[:, bk, hs],
                                    ck_nat[:, bk, hs],
                                    start=(i == 0), stop=(i == 3))
                            nc.vector.tensor_add(cg_run[hs, :],
                                                 cg_run[hs, :], dl[:])
                        nc.scalar.copy(cg_sb[:, jq - 1, :], cg_run[:])

            # ---------- phase 2: attention + output projection ----------
            with (
                tc.tile_pool(name="qkt", bufs=6) as qktpool,
                tc.tile_pool(name="sqt", bufs=4) as sqtpool,
                tc.tile_pool(name="squ", bufs=3) as squpool,
                tc.tile_pool(name="rbc", bufs=3) as rbcpool,
                tc.tile_pool(name="den", bufs=3) as denpool,
                tc.tile_pool(name="pqkt", bufs=2, space="PSUM") as pqkt,
                tc.tile_pool(name="pout", bufs=1, space="PSUM") as pout,
                tc.tile_pool(name="ost", bufs=3) as opool,
                tc.tile_pool(name="pqkv", bufs=3, space="PSUM") as pqkv,
                tc.tile_pool(name="psml", bufs=1, space="PSUM") as psml,
            ):
                yt_hi = bpool.tile([128, L], F32R, tag="vs_hi")
                qksq = bpool.tile([PH, L], F32, tag="vs_lo")
                recip = bpool.tile([PH, L], F32, tag="kscan")
                qkv_keep = {}
                for jq in range(NQ):
                    qs = slice(QC * jq, QC * (jq + 1))
                    for h in range(HPC):
                        hs = slice(32 * h, 32 * h + F)
                        qT = qt_sb[hs, qs]
                        qkv_ps = pqkv.tile([64, QC], F32, name="qkvp",
                                           tag="qkv")
                        s_ps = psml.tile([1, QC], F32, name="sp", tag="sps")
                        first_qkv = True
                        first_s = True
                        if jq > 0:
                            # history: qKV += G^T q ; s += (C q) . q
                            nc.tensor.matmul(
                                qkv_ps[:], cg_sb[hs, jq - 1, 0:DH], qT,
                                start=True, stop=False)
                            first_qkv = False
                            u_ps = pqkt.tile([F, QC], F32, name="up",
                                             tag="qkps")
                            nc.tensor.matmul(
                                u_ps[:], cg_sb[hs, jq - 1, DH:DH + F], qT,
                                start=True, stop=True)
                            squ = squpool.tile([F, QC], F32R, tag="squ")
                            nc.vector.tensor_mul(squ[:], u_ps[:],
                                                 qT.bitcast(F32))
                            nc.tensor.matmul(s_ps[:], ones_sc[0:F, :], squ[:],
                                             start=True, stop=False)
                            first_s = False
                        for g in range(4):
                            bk = 4 * jq + g
                            colr = slice(KB * g, QC)
                            qcr = slice(QC * jq + KB * g, QC * (jq + 1))
                            qk_ps = pqkt.tile([128, QC], F32, name="qkp",
                                              tag="qkps")
                            nc.tensor.matmul(
                                qk_ps[:, colr],
                                kt_sb[hs, 128 * bk:128 * (bk + 1)],
                                qt_sb[hs, qcr], start=True, stop=True)
                            qk_sbt = qktpool.tile([128, QC], F32R, tag="qksb")
                            nc.vector.tensor_mul(qk_sbt[:, colr],
                                                 qk_ps[:, colr],
                                                 masks[g][:, colr])
                            sq_t = sqtpool.tile([128, QC], F32R, tag="sqt")
                            nc.scalar.square(sq_t[:, colr],
                                             qk_sbt[:, colr].bitcast(F32))
                            nc.tensor.matmul(
                                s_ps[:, colr], ones_sc[0:128, :],
                                sq_t[:, colr], start=first_s, stop=(g == 3))
                            first_s = False
                            nc.tensor.matmul(
                                qkv_ps[:, colr],
                                cv_nat[:, bk, DH * h:DH * (h + 1)],
                                qk_sbt[:, colr], start=first_qkv,
                                stop=(g == 3))
                            first_qkv = False
                        # scan s for this head (chained along jq)
                        hr = slice(32 * h, 32 * h + 1)
                        init = (0.0 if jq == 0
                                else qksq[hr, QC * jq - 1:QC * jq])
                        nc.vector.tensor_tensor_scan(
                            qksq[hr, qs], s_ps[:],
                            masks[0][32 * h:32 * h + 1, 0:QC],
                            init, ADD, BYPASS)
                        qkv_keep[h] = qkv_ps
                    den96 = denpool.tile([PH, QC], F32, name="den96",
                                         tag="den")
                    nc.vector.tensor_add(den96[:], qksq[:, qs],
                                         nv8_96[:, qs])
                    rec96 = denpool.tile([PH, QC], F32, name="rec96",
                                         tag="rec")
                    nc.vector.reciprocal_approx_fast(out=rec96[:],
                                                     in_=den96[:])
                    for h in range(HPC):
                        rtmp = rbcpool.tile([1, QC], F32, name="rtmp",
                                            tag="rtmp")
                        nc.scalar.copy(rtmp[:],
                                       rec96[32 * h:32 * h + 1, :])
                        rbc = rbcpool.tile([64, QC], F32, tag="rbc")
                        nc.gpsimd.partition_broadcast(rbc[:], rtmp[0:1, :])
                        dst = (yt_hi[64 * h:64 * (h + 1), qs] if h < 2
                               else yt_lo[:, qs])
                        mv = (mv_hi[64 * h:64 * (h + 1), qs] if h < 2
                              else mv_lo[:, qs])
                        nc.vector.tensor_mul(dst, qkv_keep[h][:], rbc[:])
                        nc.vector.tensor_add(dst, dst.bitcast(F32), mv)
                    # ---------- output projection for this chunk ----------
                    for lb in range(4 * jq, 4 * (jq + 1)):
                        ls = slice(128 * lb, 128 * (lb + 1))
                        op_ps = pout.tile([128, D], F32, name="opp", tag="op")
                        for n0, n1 in ((0, 512), (512, 768)):
                            nc.tensor.matmul(op_ps[:, n0:n1], yt_hi[:, ls],
                                             wo_hi[:, n0:n1],
                                             start=True, stop=False)
                            nc.tensor.matmul(op_ps[:, n0:n1], yt_lo[:, ls],
                                             wo_lo[:, n0:n1],
                                             start=False, stop=True)
                        o_sb = opool.tile([128, D], F32, tag="ost")
                        nc.scalar.copy(o_sb[:], op_ps[:])
                        nc.sync.dma_start(out_e[ls, :], o_sb[:])

    nc.compile()
    return nc


_CACHED = {}


def _shard_inputs(hidden_states, Wq, Wk, Wv, Wo):
    n = np.arange(1, L + 1, dtype=np.float32)
    nv8 = np.ascontiguousarray(np.broadcast_to(8.0 * n, (PH, L)))
    invn = np.ascontiguousarray(np.broadcast_to(1.0 / n, (128, L)))
    cstv = np.full((128, 1), INV2DH8, dtype=np.float32)

    def pad_heads(w):
        out = np.zeros((D, PH), dtype=np.float32)
        for h in range(HPC):
            out[:, 32 * h:32 * h + F] = w[:, F * h:F * (h + 1)]
        return out

    in_maps = []
    for c in range(NCORES):
        b, hg = c // 4, c % 4
        hs = slice(HPC * F * hg, HPC * F * (hg + 1))
        vs = slice(HPC * DH * hg, HPC * DH * (hg + 1))
        in_maps.append({
            "hT": np.ascontiguousarray(hidden_states[b].T).astype(np.float32),
            "wq": pad_heads(np.asarray(Wq[:, hs], dtype=np.float32)),
            "wk": pad_heads(np.asarray(Wk[:, hs], dtype=np.float32)),
            "wv": np.ascontiguousarray(Wv[:, vs]).astype(np.float32),
            "wo": np.ascontiguousarray(Wo[vs, :]).astype(np.float32),
            "nv8": nv8,
            "invn": invn,
            "cst": cstv,
        })
    return in_maps


def kernel(hidden_states, Wq, Wk, Wv, Wo, _trace=False):
    from concourse.bass_utils import run_bass_kernel_spmd
    if "nc" not in _CACHED:
        _CACHED["nc"] = build_nc()
    in_maps = _shard_inputs(np.asarray(hidden_states), np.asarray(Wq),
                            np.asarray(Wk), np.asarray(Wv), np.asarray(Wo))
    res = run_bass_kernel_spmd(_CACHED["nc"], in_maps,
                               core_ids=list(range(NCORES)), trace=_trace)
    out = np.zeros((B, L, D), dtype=np.float32)
    for c in range(NCORES):
        out[c // 4] += res.results[c]["out"]
    if _trace:
        kernel._last_exec_time_ns = res.exec_time_ns
        kernel._last_profile = res
    return out


# revision 42
# speedup vs baseline: 1.0113x; 1.0113x over previous
"""Trainium2 Bass kernel for causal linear-attention approximation.

Reference computation (per batch b, head h):
  q,k = hidden @ Wq|Wk -> (L, F=16);  v = hidden @ Wv -> (L, DH=64)
  ck = k - cummean(k);  cv = v - cummean(v)        (cumsums over seq)
  qK[i,j] = q_i . ck_j   (causal: j<=i)
  s[i] = sum_j qK[i,j]^2 / (2*DH);  qKsq = cumsum_i(s);  den = (i+1)+qKsq
  y = cummean(v) + (qK @ cv) / (sqrt(DH) * den)
  out = concat_heads(y) @ Wo

Distribution: 8 cores = 2 batches x 4 head-groups (3 heads each). Each core
computes a partial (L, D) output = y_heads @ Wo_rows; host sums 4 partials
per batch.

Causal-block decomposition: for query chunk jq (512 queries), keys in earlier
chunks contribute only through running 16x16 covariance C = ck^T ck and 16x64
cross G = ck^T cv; only the 4 diagonal key blocks need explicit qK tiles.

Layout note: every per-head partition offset is 32-aligned (head h lives at
partitions [32h, 32h+16) with 16 pad rows) — walrus rejects non-32-aligned
partition bases on compute-engine APs.

Phase 1 (prefix) is emitted chunk-major so projections (PE), scans/centering
(DVE), psum copies (ACT/DVE) and transposes pipeline against each other.
"""

import numpy as np

import concourse.bacc as bacc
import concourse.mybir as mybir
import concourse.tile as tile
from concourse.masks import make_identity

F32 = mybir.dt.float32
F32R = mybir.dt.float32r
ADD = mybir.AluOpType.add
BYPASS = mybir.AluOpType.bypass

B, L, D = 2, 2048, 768
H, F, DH = 12, 16, 64
HPC = 3                 # heads per core
NCORES = 8
NB = L // 128           # 16 key blocks
NQ = L // 512           # 4 query chunks
QC = 512                # query chunk size
KB = 128                # key block size
PH = 96                 # padded per-head partition span (3 heads x 32)
INV2DH8 = 8.0 / (2.0 * DH)   # s-reduce weight: folds 1/(2*DH) and the x8

# epilogue recip broadcast: "dve" = stride-0 partition AP on DVE (fast path),
# "gpsimd" = partition_broadcast via a staged base-0 row (validated fallback)
BCAST_MODE = "gpsimd"


def build_nc():
    nc = bacc.Bacc("TRN2", target_bir_lowering=False, debug=False)

    hT = nc.declare_dram_parameter("hT", [D, L], F32, isOutput=False)
    # wq/wk padded: head h at columns [32h, 32h+16), zeros elsewhere
    wq = nc.declare_dram_parameter("wq", [D, PH], F32, isOutput=False)
    wk = nc.declare_dram_parameter("wk", [D, PH], F32, isOutput=False)
    wv = nc.declare_dram_parameter("wv", [D, HPC * DH], F32, isOutput=False)
    wo = nc.declare_dram_parameter("wo", [HPC * DH, D], F32, isOutput=False)
    nv8 = nc.declare_dram_parameter("nv8", [PH, L], F32, isOutput=False)
    invn = nc.declare_dram_parameter("invn", [128, L], F32, isOutput=False)
    cst = nc.declare_dram_parameter("cst", [128, 1], F32, isOutput=False)
    out_e = nc.declare_dram_parameter("out", [L, D], F32, isOutput=True)

    with tile.TileContext(nc) as tc:
        with (
            tc.tile_pool(name="const", bufs=1) as cpool,
            tc.tile_pool(name="wout", bufs=1) as wopool,
            tc.tile_pool(name="big", bufs=1) as bpool,
        ):
            # ---------- persistent big tiles ----------
            qt_sb = bpool.tile([PH, L], F32R, tag="qt_sb")
            kt_sb = bpool.tile([PH, L], F32R, tag="kt_sb")
            kscan = bpool.tile([PH, L], F32, tag="kscan")      # raw k cumsum
            vt_hi = bpool.tile([128, L], F32, tag="vt_hi")     # vT -> cvT
            vt_lo = bpool.tile([64, L], F32, tag="vt_lo")
            vs_hi = bpool.tile([128, L], F32, tag="vs_hi")     # raw v cumsum
            vs_lo = bpool.tile([64, L], F32, tag="vs_lo")
            mv_hi = bpool.tile([128, L], F32, tag="mv_hi")     # mean_vT
            mv_lo = bpool.tile([64, L], F32, tag="mv_lo")
            cv_nat = bpool.tile([128, NB, HPC * DH], F32R, tag="cv_nat")
            ck_nat = bpool.tile([128, NB, PH], F32R, tag="ck_nat")
            cg_run = bpool.tile([PH, DH + F], F32, tag="cg_run")
            cg_sb = bpool.tile([PH, 3, DH + F], F32R, tag="cg_sb")
            yt_lo = bpool.tile([64, L], F32R, tag="yt_lo")

            # ---------- phase 1: proj + center + transpose + C/G ----------
            with (
                tc.tile_pool(name="wproj", bufs=1) as wppool,
                tc.tile_pool(name="ht", bufs=7) as htpool,
                tc.tile_pool(name="ktmp", bufs=3) as ktmppool,
                tc.tile_pool(name="pp", bufs=1, space="PSUM") as pp,
                tc.tile_pool(name="ptr", bufs=1, space="PSUM") as ptr,
                tc.tile_pool(name="pcg", bufs=2, space="PSUM") as pcg,
            ):
                wq_sb = wppool.tile([128, 6, PH], F32R)
                nc.sync.dma_start(
                    wq_sb[:],
                    wq[:].rearrange("(c p) f -> p c f", p=128).bitcast(F32R))
                wk_sb = wppool.tile([128, 6, PH], F32R)
                nc.sync.dma_start(
                    wk_sb[:],
                    wk[:].rearrange("(c p) f -> p c f", p=128).bitcast(F32R))
                wv_sb = wppool.tile([128, 6, HPC * DH], F32R)
                nc.sync.dma_start(
                    wv_sb[:],
                    wv[:].rearrange("(c p) f -> p c f", p=128).bitcast(F32R))

                hts0 = []
                for k in range(6):
                    ht0 = htpool.tile([128, QC], F32R, name="ht", tag="ht")
                    hts0.append(ht0)
                    nc.scalar.dma_start(
                        ht0[:], hT[128 * k:128 * (k + 1), 0:QC].bitcast(F32R))

                # constants (loaded after the critical-path weight DMAs)
                idt = cpool.tile([128, 128], F32)
                make_identity(nc, idt[:])
                ones_sc = cpool.tile([128, 1], F32R)
                nc.scalar.dma_start(ones_sc[:], cst[:].bitcast(F32R))
                invn_row = cpool.tile([1, L], F32)
                nc.scalar.dma_start(invn_row[:], invn[0:1, :])
                invn_bc = cpool.tile([128, L], F32)
                nc.gpsimd.partition_broadcast(invn_bc[:], invn_row[0:1, :])
                nv8_row = cpool.tile([1, L], F32)
                nc.scalar.dma_start(nv8_row[:], nv8[0:1, :])
                nv8_96 = cpool.tile([PH, L], F32)
                nc.gpsimd.partition_broadcast(nv8_96[:], nv8_row[0:1, :])
                masks = []
                for g in range(4):
                    m = cpool.tile([128, QC], F32, name=f"mask{g}",
                                   tag=f"mask{g}")
                    nc.gpsimd.memset(m[:], 1.0)
                    nc.gpsimd.affine_select(
                        out=m[:], in_=m[:], compare_op=mybir.AluOpType.is_ge,
                        fill=0.0, base=-128 * g, pattern=[[1, QC]],
                        channel_multiplier=-1,
                    )
                    masks.append(m)
                wo_hi = wopool.tile([128, D], F32R)
                wo_lo = wopool.tile([64, D], F32R)
                nc.scalar.dma_start(wo_hi[:], wo[0:128, :].bitcast(F32R))
                nc.scalar.dma_start(wo_lo[:], wo[128:192, :].bitcast(F32R))

                for jq in range(NQ):
                    qs = slice(QC * jq, QC * (jq + 1))
                    # --- projections for this chunk (two psum sub-passes) ---
                    hts = []
                    p_q = pp.tile([PH, QC], F32, name="psq", tag="pq")
                    p_k = pp.tile([PH, QC], F32, name="psk", tag="pk")
                    for k in range(6):
                        if jq == 0:
                            ht_t = hts0[k]
                        else:
                            ht_t = htpool.tile([128, QC], F32R, name="ht",
                                               tag="ht")
                            nc.sync.dma_start(
                                ht_t[:],
                                hT[128 * k:128 * (k + 1), qs].bitcast(F32R))
                        hts.append(ht_t)
                        st, sp = (k == 0), (k == 5)
                        nc.tensor.matmul(p_q[:], wq_sb[:, k, :], ht_t[:],
                                         start=st, stop=sp)
                        nc.tensor.matmul(p_k[:], wk_sb[:, k, :], ht_t[:],
                                         start=st, stop=sp)
                    nc.scalar.copy(qt_sb[:, qs], p_q[:])
                    nc.scalar.copy(kt_sb[:, qs], p_k[:])
                    p_vh = pp.tile([128, QC], F32, name="psvh", tag="pq")
                    p_vl = pp.tile([64, QC], F32, name="psvl", tag="pk")
                    for k in range(6):
                        st, sp = (k == 0), (k == 5)
                        nc.tensor.matmul(p_vh[:], wv_sb[:, k, 0:128],
                                         hts[k][:], start=st, stop=sp)
                        nc.tensor.matmul(p_vl[:], wv_sb[:, k, 128:192],
                                         hts[k][:], start=st, stop=sp)
                    nc.scalar.copy(vt_hi[:, qs], p_vh[:])
                    nc.scalar.copy(vt_lo[:, qs], p_vl[:])

                    # --- centering for this chunk (chained scans) ---
                    ik = (0.0 if jq == 0 else kscan[:, QC * jq - 1:QC * jq])
                    nc.vector.tensor_tensor_scan(
                        kscan[:, qs], kt_sb[:, qs].bitcast(F32),
                        kt_sb[:, qs].bitcast(F32), ik, ADD, BYPASS)
                    ktmp = ktmppool.tile([PH, QC], F32, name="ktmp",
                                         tag="ktmp")
                    nc.vector.tensor_mul(ktmp[:], kscan[:, qs],
                                         invn_bc[0:PH, qs])
                    nc.vector.tensor_sub(kt_sb[:, qs],
                                         kt_sb[:, qs].bitcast(F32), ktmp[:])
                    ih = (0.0 if jq == 0 else vs_hi[:, QC * jq - 1:QC * jq])
                    nc.vector.tensor_tensor_scan(
                        vs_hi[:, qs], vt_hi[:, qs], vt_hi[:, qs],
                        ih, ADD, BYPASS)
                    il = (0.0 if jq == 0 else vs_lo[:, QC * jq - 1:QC * jq])
                    nc.vector.tensor_tensor_scan(
                        vs_lo[:, qs], vt_lo[:, qs], vt_lo[:, qs],
                        il, ADD, BYPASS)
                    nc.vector.tensor_mul(mv_hi[:, qs], vs_hi[:, qs],
                                         invn_bc[0:128, qs])
                    nc.vector.tensor_mul(mv_lo[:, qs], vs_lo[:, qs],
                                         invn_bc[0:64, qs])
                    nc.vector.tensor_sub(vt_hi[:, qs], vt_hi[:, qs],
                                         mv_hi[:, qs])   # cvT
                    nc.vector.tensor_sub(vt_lo[:, qs], vt_lo[:, qs],
                                         mv_lo[:, qs])   # cvT

                    # --- transposes for this chunk's 4 key blocks ---
                    for lb in range(4 * jq, 4 * (jq + 1)):
                        cs = slice(128 * lb, 128 * (lb + 1))
                        tch = ptr.tile([128, 128], F32, name="tch", tag="tch")
                        nc.tensor.transpose(tch[:], vt_hi[:, cs], idt[:])
                        nc.scalar.copy(cv_nat[:, lb, 0:128], tch[:])
                        tcl = ptr.tile([128, 64], F32, name="tcl", tag="tcl")
                        nc.tensor.transpose(tcl[:], vt_lo[:, cs],
                                            idt[0:64, 0:64])
                        nc.scalar.copy(cv_nat[:, lb, 128:192], tcl[:])
                        tck = ptr.tile([128, PH], F32, name="tck", tag="tck")
                        nc.tensor.transpose(tck[:], kt_sb[:, cs].bitcast(F32),
                                            idt[0:PH, 0:PH])
                        nc.scalar.copy(ck_nat[:, lb, :], tck[:])

                    # --- C/G prefix snapshot (covers blocks of chunk jq-1) ---
                    if jq == 0:
                        nc.vector.memset(cg_run[:], 0.0)
                    else:
                        for h in range(HPC):
                            hs = slice(32 * h, 32 * h + F)
                            dl = pcg.tile([F, DH + F], F32, name="dl",
                                          tag="cgd")
                            for i in range(4):
                                bk = 4 * (jq - 1) + i
                                nc.tensor.matmul(
                                    dl[:, 0:DH],
                                    ck_nat[:, bk, hs],
                                    cv_nat[:, bk, DH * h:DH * (h + 1)],
                                    start=(i == 0), stop=(i == 3))
                                nc.tensor.matmul(
                                    dl[:, DH:DH + F],
                                    ck_nat[:, bk, hs],
                                    ck_nat[:, bk, hs],
                                    start=(i == 0), stop=(i == 3))
                            nc.vector.tensor_add(cg_run[hs, :],
                                                 cg_run[hs, :], dl[:])
                        nc.scalar.copy(cg_sb[:, jq - 1, :], cg_run[:])

            # ---------- phase 2: attention + output projection ----------
            with (
                tc.tile_pool(name="qkt", bufs=6) as qktpool,
                tc.tile_pool(name="sqt", bufs=4) as sqtpool,
                tc.tile_pool(name="squ", bufs=3) as squpool,
                tc.tile_pool(name="rbc", bufs=3) as rbcpool,
                tc.tile_pool(name="den", bufs=3) as denpool,
                tc.tile_pool(name="pqkt", bufs=2, space="PSUM") as pqkt,
                tc.tile_pool(name="pout", bufs=1, space="PSUM") as pout,
                tc.tile_pool(name="ost", bufs=3) as opool,
                tc.tile_pool(name="pqkv", bufs=3, space="PSUM") as pqkv,
                tc.tile_pool(name="psml", bufs=1, space="PSUM") as psml,
            ):
                yt_hi = bpool.tile([128, L], F32R, tag="vs_hi")
                qksq = bpool.tile([PH, L], F32, tag="vs_lo")
                recip = bpool.tile([PH, L], F32, tag="kscan")
                qkv_keep = {}
                for jq in range(NQ):
                    qs = slice(QC * jq, QC * (jq + 1))
                    for h in range(HPC):
                        hs = slice(32 * h, 32 * h + F)
                        qT = qt_sb[hs, qs]
                        qkv_ps = pqkv.tile([64, QC], F32, name="qkvp",
                                           tag="qkv")
                        s_ps = psml.tile([1, QC], F32, name="sp", tag="sps")
                        first_qkv = True
                        first_s = True
                        if jq > 0:
                            # history: qKV += G^T q ; s += (C q) . q
                            nc.tensor.matmul(
                                qkv_ps[:], cg_sb[hs, jq - 1, 0:DH], qT,
                                start=True, stop=False)
                            first_qkv = False
                            u_ps = pqkt.tile([F, QC], F32, name="up",
                                             tag="qkps")
                            nc.tensor.matmul(
                                u_ps[:], cg_sb[hs, jq - 1, DH:DH + F], qT,
                                start=True, stop=True)
                            squ = squpool.tile([F, QC], F32R, tag="squ")
                            nc.vector.tensor_mul(squ[:], u_ps[:],
                                                 qT.bitcast(F32))
                            nc.tensor.matmul(s_ps[:], ones_sc[0:F, :], squ[:],
                                             start=True, stop=False)
                            first_s = False
                        for g in range(4):
                            bk = 4 * jq + g
                            colr = slice(KB * g, QC)
                            qcr = slice(QC * jq + KB * g, QC * (jq + 1))
                            qk_ps = pqkt.tile([128, QC], F32, name="qkp",
                                              tag="qkps")
                            nc.tensor.matmul(
                                qk_ps[:, colr],
                                kt_sb[hs, 128 * bk:128 * (bk + 1)],
                                qt_sb[hs, qcr], start=True, stop=True)
                            qk_sbt = qktpool.tile([128, QC], F32R, tag="qksb")
                            nc.vector.tensor_mul(qk_sbt[:, colr],
                                                 qk_ps[:, colr],
                                                 masks[g][:, colr])
                            sq_t = sqtpool.tile([128, QC], F32R, tag="sqt")
                            nc.scalar.square(sq_t[:, colr],
                                             qk_sbt[:, colr].bitcast(F32))
                            nc.tensor.matmul(
                                s_ps[:, colr], ones_sc[0:128, :],
                                sq_t[:, colr], start=first_s, stop=(g == 3))
                            first_s = False
                            nc.tensor.matmul(
                                qkv_ps[:, colr],
                                cv_nat[:, bk, DH * h:DH * (h + 1)],
                                qk_sbt[:, colr], start=first_qkv,
                                stop=(g == 3))
                            first_qkv = False
                        # scan s for this head (chained along jq)
                        hr = slice(32 * h, 32 * h + 1)
                        init = (0.0 if jq == 0
                                else qksq[hr, QC * jq - 1:QC * jq])
                        nc.vector.tensor_tensor_scan(
                            qksq[hr, qs], s_ps[:],
                            masks[0][32 * h:32 * h + 1, 0:QC],
                            init, ADD, BYPASS)
                        qkv_keep[h] = qkv_ps
                    den96 = denpool.tile([PH, QC], F32, name="den96",
                                         tag="den")
                    nc.vector.tensor_add(den96[:], qksq[:, qs],
                                         nv8_96[:, qs])
                    rec96 = denpool.tile([PH, QC], F32, name="rec96",
                                         tag="rec")
                    nc.vector.reciprocal_approx_fast(out=rec96[:],
                                                     in_=den96[:])
                    for h in range(HPC):
                        rtmp = rbcpool.tile([1, QC], F32, name="rtmp",
                                            tag="rtmp")
                        nc.scalar.copy(rtmp[:],
                                       rec96[32 * h:32 * h + 1, :])
                        rbc = rbcpool.tile([64, QC], F32, tag="rbc")
                        nc.gpsimd.partition_broadcast(rbc[:], rtmp[0:1, :])
                        dst = (yt_hi[64 * h:64 * (h + 1), qs] if h < 2
                               else yt_lo[:, qs])
                        mv = (mv_hi[64 * h:64 * (h + 1), qs] if h < 2
                              else mv_lo[:, qs])
                        nc.vector.tensor_mul(dst, qkv_keep[h][:], rbc[:])
                        nc.vector.tensor_add(dst, dst.bitcast(F32), mv)
                    # ---------- output projection for this chunk ----------
                    for lb in range(4 * jq, 4 * (jq + 1)):
                        ls = slice(128 * lb, 128 * (lb + 1))
                        op_ps = pout.tile([128, D], F32, name="opp", tag="op")
                        for n0, n1 in ((0, 512), (512, 768)):
                            nc.tensor.matmul(op_ps[:, n0:n1], yt_hi[:, ls],
                                             wo_hi[:, n0:n1],
                                             start=True, stop=False)
                            nc.tensor.matmul(op_ps[:, n0:n1], yt_lo[:, ls],
                                             wo_lo[:, n0:n1],
                                             start=False, stop=True)
                        o_sb = opool.tile([128, D], F32, tag="ost")
                        nc.scalar.copy(o_sb[:], op_ps[:])
                        nc.sync.dma_start(out_e[ls, :], o_sb[:])

    nc.compile()
    return nc


_CACHED = {}


def _shard_inputs(hidden_states, Wq, Wk, Wv, Wo):
    n = np.arange(1, L + 1, dtype=np.float32)
    nv8 = np.ascontiguousarray(np.broadcast_to(8.0 * n, (PH, L)))
    invn = np.ascontiguousarray(np.broadcast_to(1.0 / n, (128, L)))
    cstv = np.full((128, 1), INV2DH8, dtype=np.float32)

    def pad_heads(w):
        out = np.zeros((D, PH), dtype=np.float32)
        for h in range(HPC):
            out[:, 32 * h:32 * h + F] = w[:, F * h:F * (h + 1)]
        return out

    in_maps = []
    for c in range(NCORES):
        b, hg = c // 4, c % 4
        hs = slice(HPC * F * hg, HPC * F * (hg + 1))
        vs = slice(HPC * DH * hg, HPC * DH * (hg + 1))
        in_maps.append({
            "hT": np.ascontiguousarray(hidden_states[b].T).astype(np.float32),
            "wq": pad_heads(np.asarray(Wq[:, hs], dtype=np.float32)),
            "wk": pad_heads(np.asarray(Wk[:, hs], dtype=np.float32)),
            "wv": np.ascontiguousarray(Wv[:, vs]).astype(np.float32),
            "wo": np.ascontiguousarray(Wo[vs, :]).astype(np.float32),
            "nv8": nv8,
            "invn": invn,
            "cst": cstv,
        })
    return in_maps


def kernel(hidden_states, Wq, Wk, Wv, Wo, _trace=False):
    from concourse.bass_utils import run_bass_kernel_spmd
    if "nc" not in _CACHED:
        _CACHED["nc"] = build_nc()
    in_maps = _shard_inputs(np.asarray(hidden_states), np.asarray(Wq),
                            np.asarray(Wk), np.asarray(Wv), np.asarray(Wo))
    res = run_bass_kernel_spmd(_CACHED["nc"], in_maps,
                               core_ids=list(range(NCORES)), trace=_trace)
    out = np.zeros((B, L, D), dtype=np.float32)
    for c in range(NCORES):
        out[c // 4] += res.results[c]["out"]
    if _trace:
        kernel._last_exec_time_ns = res.exec_time_ns
        kernel._last_profile = res
    return out


# revision 44
# speedup vs baseline: 1.0234x; 1.0119x over previous
"""Trainium2 Bass kernel for causal linear-attention approximation.

Reference computation (per batch b, head h):
  q,k = hidden @ Wq|Wk -> (L, F=16);  v = hidden @ Wv -> (L, DH=64)
  ck = k - cummean(k);  cv = v - cummean(v)        (cumsums over seq)
  qK[i,j] = q_i . ck_j   (causal: j<=i)
  s[i] = sum_j qK[i,j]^2 / (2*DH);  qKsq = cumsum_i(s);  den = (i+1)+qKsq
  y = cummean(v) + (qK @ cv) / (sqrt(DH) * den)
  out = concat_heads(y) @ Wo

Distribution: 8 cores = 2 batches x 4 head-groups (3 heads each). Each core
computes a partial (L, D) output = y_heads @ Wo_rows; host sums 4 partials
per batch.

Causal-block decomposition: for query chunk jq (512 queries), keys in earlier
chunks contribute only through running 16x16 covariance C = ck^T ck and 16x64
cross G = ck^T cv; only the 4 diagonal key blocks need explicit qK tiles.

Layout note: every per-head partition offset is 32-aligned (head h lives at
partitions [32h, 32h+16) with 16 pad rows) — walrus rejects non-32-aligned
partition bases on compute-engine APs.

Phase 1 (prefix) is emitted chunk-major so projections (PE), scans/centering
(DVE), psum copies (ACT/DVE) and transposes pipeline against each other.
"""

import numpy as np

import concourse.bacc as bacc
import concourse.mybir as mybir
import concourse.tile as tile
from concourse.masks import make_identity

F32 = mybir.dt.float32
F32R = mybir.dt.float32r
ADD = mybir.AluOpType.add
BYPASS = mybir.AluOpType.bypass

B, L, D = 2, 2048, 768
H, F, DH = 12, 16, 64
HPC = 3                 # heads per core
NCORES = 8
NB = L // 128           # 16 key blocks
NQ = L // 512           # 4 query chunks
QC = 512                # query chunk size
KB = 128                # key block size
PH = 96                 # padded per-head partition span (3 heads x 32)
INV2DH8 = 8.0 / (2.0 * DH)   # s-reduce weight: folds 1/(2*DH) and the x8

# epilogue recip broadcast: "dve" = stride-0 partition AP on DVE (fast path),
# "gpsimd" = partition_broadcast via a staged base-0 row (validated fallback)
BCAST_MODE = "gpsimd"


def build_nc():
    nc = bacc.Bacc("TRN2", target_bir_lowering=False, debug=False)

    hT = nc.declare_dram_parameter("hT", [D, L], F32, isOutput=False)
    # wq/wk padded: head h at columns [32h, 32h+16), zeros elsewhere
    wq = nc.declare_dram_parameter("wq", [D, PH], F32, isOutput=False)
    wk = nc.declare_dram_parameter("wk", [D, PH], F32, isOutput=False)
    wv = nc.declare_dram_parameter("wv", [D, HPC * DH], F32, isOutput=False)
    wo = nc.declare_dram_parameter("wo", [HPC * DH, D], F32, isOutput=False)
    nv8 = nc.declare_dram_parameter("nv8", [PH, L], F32, isOutput=False)
    invn = nc.declare_dram_parameter("invn", [128, L], F32, isOutput=False)
    cst = nc.declare_dram_parameter("cst", [128, 1], F32, isOutput=False)
    out_e = nc.declare_dram_parameter("out", [L, D], F32, isOutput=True)

    with tile.TileContext(nc) as tc:
        with (
            tc.tile_pool(name="const", bufs=1) as cpool,
            tc.tile_pool(name="wout", bufs=1) as wopool,
            tc.tile_pool(name="big", bufs=1) as bpool,
        ):
            # ---------- persistent big tiles ----------
            qt_sb = bpool.tile([PH, L], F32R, tag="qt_sb")
            kt_sb = bpool.tile([PH, L], F32R, tag="kt_sb")
            kscan = bpool.tile([PH, L], F32, tag="kscan")      # raw k cumsum
            vt_hi = bpool.tile([128, L], F32, tag="vt_hi")     # vT -> cvT
            vt_lo = bpool.tile([64, L], F32, tag="vt_lo")
            vs_hi = bpool.tile([128, L], F32, tag="vs_hi")     # raw v cumsum
            vs_lo = bpool.tile([64, L], F32, tag="vs_lo")
            mv_hi = bpool.tile([128, L], F32, tag="mv_hi")     # mean_vT
            mv_lo = bpool.tile([64, L], F32, tag="mv_lo")
            cv_nat = bpool.tile([128, NB, HPC * DH], F32R, tag="cv_nat")
            ck_nat = bpool.tile([128, NB, PH], F32R, tag="ck_nat")
            cg_run = bpool.tile([PH, DH + F], F32, tag="cg_run")
            cg_sb = bpool.tile([PH, 3, DH + F], F32R, tag="cg_sb")
            yt_lo = bpool.tile([64, L], F32R, tag="yt_lo")

            # ---------- phase 1: proj + center + transpose + C/G ----------
            with (
                tc.tile_pool(name="wproj", bufs=1) as wppool,
                tc.tile_pool(name="ht", bufs=8) as htpool,
                tc.tile_pool(name="ktmp", bufs=2) as ktmppool,
                tc.tile_pool(name="pp", bufs=1, space="PSUM") as pp,
                tc.tile_pool(name="ptr", bufs=1, space="PSUM") as ptr,
                tc.tile_pool(name="pcg", bufs=2, space="PSUM") as pcg,
            ):
                wq_sb = wppool.tile([128, 6, PH], F32R)
                nc.sync.dma_start(
                    wq_sb[:],
                    wq[:].rearrange("(c p) f -> p c f", p=128).bitcast(F32R))
                wk_sb = wppool.tile([128, 6, PH], F32R)
                nc.sync.dma_start(
                    wk_sb[:],
                    wk[:].rearrange("(c p) f -> p c f", p=128).bitcast(F32R))
                wv_sb = wppool.tile([128, 6, HPC * DH], F32R)
                nc.sync.dma_start(
                    wv_sb[:],
                    wv[:].rearrange("(c p) f -> p c f", p=128).bitcast(F32R))

                hts0 = []
                for k in range(6):
                    ht0 = htpool.tile([128, QC], F32R, name="ht", tag="ht")
                    hts0.append(ht0)
                    nc.scalar.dma_start(
                        ht0[:], hT[128 * k:128 * (k + 1), 0:QC].bitcast(F32R))

                # constants (loaded after the critical-path weight DMAs)
                idt = cpool.tile([128, 128], F32)
                make_identity(nc, idt[:])
                ones_sc = cpool.tile([128, 1], F32R)
                nc.scalar.dma_start(ones_sc[:], cst[:].bitcast(F32R))
                invn_row = cpool.tile([1, L], F32)
                nc.scalar.dma_start(invn_row[:], invn[0:1, :])
                invn_bc = cpool.tile([128, L], F32)
                nc.gpsimd.partition_broadcast(invn_bc[:], invn_row[0:1, :])
                nv8_row = cpool.tile([1, L], F32)
                nc.scalar.dma_start(nv8_row[:], nv8[0:1, :])
                nv8_96 = cpool.tile([PH, L], F32)
                nc.gpsimd.partition_broadcast(nv8_96[:], nv8_row[0:1, :])
                masks = []
                for g in range(4):
                    m = cpool.tile([128, QC], F32, name=f"mask{g}",
                                   tag=f"mask{g}")
                    nc.gpsimd.memset(m[:], 1.0)
                    nc.gpsimd.affine_select(
                        out=m[:], in_=m[:], compare_op=mybir.AluOpType.is_ge,
                        fill=0.0, base=-128 * g, pattern=[[1, QC]],
                        channel_multiplier=-1,
                    )
                    masks.append(m)
                wo_hi = wopool.tile([128, D], F32R)
                wo_lo = wopool.tile([64, D], F32R)
                nc.scalar.dma_start(wo_hi[:], wo[0:128, :].bitcast(F32R))
                nc.scalar.dma_start(wo_lo[:], wo[128:192, :].bitcast(F32R))

                for jq in range(NQ):
                    qs = slice(QC * jq, QC * (jq + 1))
                    # --- projections for this chunk (two psum sub-passes) ---
                    hts = []
                    p_q = pp.tile([PH, QC], F32, name="psq", tag="pq")
                    p_k = pp.tile([PH, QC], F32, name="psk", tag="pk")
                    for k in range(6):
                        if jq == 0:
                            ht_t = hts0[k]
                        else:
                            ht_t = htpool.tile([128, QC], F32R, name="ht",
                                               tag="ht")
                            nc.sync.dma_start(
                                ht_t[:],
                                hT[128 * k:128 * (k + 1), qs].bitcast(F32R))
                        hts.append(ht_t)
                        st, sp = (k == 0), (k == 5)
                        nc.tensor.matmul(p_q[:], wq_sb[:, k, :], ht_t[:],
                                         start=st, stop=sp)
                        nc.tensor.matmul(p_k[:], wk_sb[:, k, :], ht_t[:],
                                         start=st, stop=sp)
                    nc.scalar.copy(qt_sb[:, qs], p_q[:])
                    nc.scalar.copy(kt_sb[:, qs], p_k[:])
                    p_vh = pp.tile([128, QC], F32, name="psvh", tag="pq")
                    p_vl = pp.tile([64, QC], F32, name="psvl", tag="pk")
                    for k in range(6):
                        st, sp = (k == 0), (k == 5)
                        nc.tensor.matmul(p_vh[:], wv_sb[:, k, 0:128],
                                         hts[k][:], start=st, stop=sp)
                        nc.tensor.matmul(p_vl[:], wv_sb[:, k, 128:192],
                                         hts[k][:], start=st, stop=sp)
                    nc.scalar.copy(vt_hi[:, qs], p_vh[:])
                    nc.scalar.copy(vt_lo[:, qs], p_vl[:])

                    # --- centering for this chunk (chained scans) ---
                    ik = (0.0 if jq == 0 else kscan[:, QC * jq - 1:QC * jq])
                    nc.vector.tensor_tensor_scan(
                        kscan[:, qs], kt_sb[:, qs].bitcast(F32),
                        kt_sb[:, qs].bitcast(F32), ik, ADD, BYPASS)
                    ktmp = ktmppool.tile([PH, QC], F32, name="ktmp",
                                         tag="ktmp")
                    nc.vector.tensor_mul(ktmp[:], kscan[:, qs],
                                         invn_bc[0:PH, qs])
                    nc.vector.tensor_sub(kt_sb[:, qs],
                                         kt_sb[:, qs].bitcast(F32), ktmp[:])
                    ih = (0.0 if jq == 0 else vs_hi[:, QC * jq - 1:QC * jq])
                    nc.vector.tensor_tensor_scan(
                        vs_hi[:, qs], vt_hi[:, qs], vt_hi[:, qs],
                        ih, ADD, BYPASS)
                    il = (0.0 if jq == 0 else vs_lo[:, QC * jq - 1:QC * jq])
                    nc.vector.tensor_tensor_scan(
                        vs_lo[:, qs], vt_lo[:, qs], vt_lo[:, qs],
                        il, ADD, BYPASS)
                    nc.vector.tensor_mul(mv_hi[:, qs], vs_hi[:, qs],
                                         invn_bc[0:128, qs])
                    nc.vector.tensor_mul(mv_lo[:, qs], vs_lo[:, qs],
                                         invn_bc[0:64, qs])
                    nc.vector.tensor_sub(vt_hi[:, qs], vt_hi[:, qs],
                                         mv_hi[:, qs])   # cvT
                    nc.vector.tensor_sub(vt_lo[:, qs], vt_lo[:, qs],
                                         mv_lo[:, qs])   # cvT

                    # --- transposes for this chunk's 4 key blocks ---
                    for lb in range(4 * jq, 4 * (jq + 1)):
                        cs = slice(128 * lb, 128 * (lb + 1))
                        tch = ptr.tile([128, 128], F32, name="tch", tag="tch")
                        nc.tensor.transpose(tch[:], vt_hi[:, cs], idt[:])
                        nc.scalar.copy(cv_nat[:, lb, 0:128], tch[:])
                        tcl = ptr.tile([128, 64], F32, name="tcl", tag="tcl")
                        nc.tensor.transpose(tcl[:], vt_lo[:, cs],
                                            idt[0:64, 0:64])
                        nc.scalar.copy(cv_nat[:, lb, 128:192], tcl[:])
                        tck = ptr.tile([128, PH], F32, name="tck", tag="tck")
                        nc.tensor.transpose(tck[:], kt_sb[:, cs].bitcast(F32),
                                            idt[0:PH, 0:PH])
                        nc.scalar.copy(ck_nat[:, lb, :], tck[:])

                    # --- C/G prefix snapshot (covers blocks of chunk jq-1) ---
                    if jq == 0:
                        nc.vector.memset(cg_run[:], 0.0)
                    else:
                        for h in range(HPC):
                            hs = slice(32 * h, 32 * h + F)
                            dl = pcg.tile([F, DH + F], F32, name="dl",
                                          tag="cgd")
                            for i in range(4):
                                bk = 4 * (jq - 1) + i
                                nc.tensor.matmul(
                                    dl[:, 0:DH],
                                    ck_nat[:, bk, hs],
                                    cv_nat[:, bk, DH * h:DH * (h + 1)],
                                    start=(i == 0), stop=(i == 3))
                                nc.tensor.matmul(
                                    dl[:, DH:DH + F],
                                    ck_nat[:, bk, hs],
                                    ck_nat[:, bk, hs],
                                    start=(i == 0), stop=(i == 3))
                            nc.vector.tensor_add(cg_run[hs, :],
                                                 cg_run[hs, :], dl[:])
                        nc.scalar.copy(cg_sb[:, jq - 1, :], cg_run[:])

            # ---------- phase 2: attention + output projection ----------
            with (
                tc.tile_pool(name="qkt", bufs=6) as qktpool,
                tc.tile_pool(name="sqt", bufs=4) as sqtpool,
                tc.tile_pool(name="squ", bufs=3) as squpool,
                tc.tile_pool(name="rbc", bufs=3) as rbcpool,
                tc.tile_pool(name="den", bufs=3) as denpool,
                tc.tile_pool(name="pqkt", bufs=2, space="PSUM") as pqkt,
                tc.tile_pool(name="pout", bufs=1, space="PSUM") as pout,
                tc.tile_pool(name="ost", bufs=3) as opool,
                tc.tile_pool(name="pqkv", bufs=3, space="PSUM") as pqkv,
                tc.tile_pool(name="psml", bufs=1, space="PSUM") as psml,
            ):
                yt_hi = bpool.tile([128, L], F32R, tag="vs_hi")
                qksq = bpool.tile([PH, L], F32, tag="vs_lo")
                recip = bpool.tile([PH, L], F32, tag="kscan")
                qkv_keep = {}
                for jq in range(NQ):
                    qs = slice(QC * jq, QC * (jq + 1))
                    for h in range(HPC):
                        hs = slice(32 * h, 32 * h + F)
                        qT = qt_sb[hs, qs]
                        qkv_ps = pqkv.tile([64, QC], F32, name="qkvp",
                                           tag="qkv")
                        s_ps = psml.tile([1, QC], F32, name="sp", tag="sps")
                        first_qkv = True
                        first_s = True
                        if jq > 0:
                            # history: qKV += G^T q ; s += (C q) . q
                            nc.tensor.matmul(
                                qkv_ps[:], cg_sb[hs, jq - 1, 0:DH], qT,
                                start=True, stop=False)
                            first_qkv = False
                            u_ps = pqkt.tile([F, QC], F32, name="up",
                                             tag="qkps")
                            nc.tensor.matmul(
                                u_ps[:], cg_sb[hs, jq - 1, DH:DH + F], qT,
                                start=True, stop=True)
                            squ = squpool.tile([F, QC], F32R, tag="squ")
                            nc.vector.tensor_mul(squ[:], u_ps[:],
                                                 qT.bitcast(F32))
                            nc.tensor.matmul(s_ps[:], ones_sc[0:F, :], squ[:],
                                             start=True, stop=False)
                            first_s = False
                        for g in range(4):
                            bk = 4 * jq + g
                            colr = slice(KB * g, QC)
                            qcr = slice(QC * jq + KB * g, QC * (jq + 1))
                            qk_ps = pqkt.tile([128, QC], F32, name="qkp",
                                              tag="qkps")
                            nc.tensor.matmul(
                                qk_ps[:, colr],
                                kt_sb[hs, 128 * bk:128 * (bk + 1)],
                                qt_sb[hs, qcr], start=True, stop=True)
                            qk_sbt = qktpool.tile([128, QC], F32R, tag="qksb")
                            nc.vector.tensor_mul(qk_sbt[:, colr],
                                                 qk_ps[:, colr],
                                                 masks[g][:, colr])
                            sq_t = sqtpool.tile([128, QC], F32R, tag="sqt")
                            nc.scalar.square(sq_t[:, colr],
                                             qk_sbt[:, colr].bitcast(F32))
                            nc.tensor.matmul(
                                s_ps[:, colr], ones_sc[0:128, :],
                                sq_t[:, colr], start=first_s, stop=(g == 3))
                            first_s = False
                            nc.tensor.matmul(
                                qkv_ps[:, colr],
                                cv_nat[:, bk, DH * h:DH * (h + 1)],
                                qk_sbt[:, colr], start=first_qkv,
                                stop=(g == 3))
                            first_qkv = False
                        # scan s for this head (chained along jq)
                        hr = slice(32 * h, 32 * h + 1)
                        init = (0.0 if jq == 0
                                else qksq[hr, QC * jq - 1:QC * jq])
                        nc.vector.tensor_tensor_scan(
                            qksq[hr, qs], s_ps[:],
                            masks[0][32 * h:32 * h + 1, 0:QC],
                            init, ADD, BYPASS)
                        qkv_keep[h] = qkv_ps
                    den96 = denpool.tile([PH, QC], F32, name="den96",
                                         tag="den")
                    nc.vector.tensor_add(den96[:], qksq[:, qs],
                                         nv8_96[:, qs])
                    rec96 = denpool.tile([PH, QC], F32, name="rec96",
                                         tag="rec")
                    nc.vector.reciprocal_approx_fast(out=rec96[:],
                                                     in_=den96[:])
                    for h in range(HPC):
                        rtmp = rbcpool.tile([1, QC], F32, name="rtmp",
                                            tag="rtmp")
                        nc.scalar.copy(rtmp[:],
                                       rec96[32 * h:32 * h + 1, :])
                        rbc = rbcpool.tile([64, QC], F32, tag="rbc")
                        nc.gpsimd.partition_broadcast(rbc[:], rtmp[0:1, :])
                        dst = (yt_hi[64 * h:64 * (h + 1), qs] if h < 2
                               else yt_lo[:, qs])
                        mv = (mv_hi[64 * h:64 * (h + 1), qs] if h < 2
                              else mv_lo[:, qs])
                        nc.vector.tensor_mul(dst, qkv_keep[h][:], rbc[:])
                        nc.vector.tensor_add(dst, dst.bitcast(F32), mv)
                    # ---------- output projection for this chunk ----------
                    for lb in range(4 * jq, 4 * (jq + 1)):
                        ls = slice(128 * lb, 128 * (lb + 1))
                        op_ps = pout.tile([128, D], F32, name="opp", tag="op")
                        for n0, n1 in ((0, 512), (512, 768)):
                            nc.tensor.matmul(op_ps[:, n0:n1], yt_hi[:, ls],
                                             wo_hi[:, n0:n1],
                                             start=True, stop=False)
                            nc.tensor.matmul(op_ps[:, n0:n1], yt_lo[:, ls],
                                             wo_lo[:, n0:n1],
                                             start=False, stop=True)
                        o_sb = opool.tile([128, D], F32, tag="ost")
                        nc.scalar.copy(o_sb[:], op_ps[:])
                        nc.sync.dma_start(out_e[ls, :], o_sb[:])

    nc.compile()
    return nc


_CACHED = {}


def _shard_inputs(hidden_states, Wq, Wk, Wv, Wo):
    n = np.arange(1, L + 1, dtype=np.float32)
    nv8 = np.ascontiguousarray(np.broadcast_to(8.0 * n, (PH, L)))
    invn = np.ascontiguousarray(np.broadcast_to(1.0 / n, (128, L)))
    cstv = np.full((128, 1), INV2DH8, dtype=np.float32)

    def pad_heads(w):
        out = np.zeros((D, PH), dtype=np.float32)
        for h in range(HPC):
            out[:, 32 * h:32 * h + F] = w[:, F * h:F * (h + 1)]
        return out

    in_maps = []
    for c in range(NCORES):
        b, hg = c // 4, c % 4
        hs = slice(HPC * F * hg, HPC * F * (hg + 1))
        vs = slice(HPC * DH * hg, HPC * DH * (hg + 1))
        in_maps.append({
            "hT": np.ascontiguousarray(hidden_states[b].T).astype(np.float32),
            "wq": pad_heads(np.asarray(Wq[:, hs], dtype=np.float32)),
            "wk": pad_heads(np.asarray(Wk[:, hs], dtype=np.float32)),
            "wv": np.ascontiguousarray(Wv[:, vs]).astype(np.float32),
            "wo": np.ascontiguousarray(Wo[vs, :]).astype(np.float32),
            "nv8": nv8,
            "invn": invn,
            "cst": cstv,
        })
    return in_maps


def kernel(hidden_states, Wq, Wk, Wv, Wo, _trace=False):
    from concourse.bass_utils import run_bass_kernel_spmd
    if "nc" not in _CACHED:
        _CACHED["nc"] = build_nc()
    in_maps = _shard_inputs(np.asarray(hidden_states), np.asarray(Wq),
                            np.asarray(Wk), np.asarray(Wv), np.asarray(Wo))
    res = run_bass_kernel_spmd(_CACHED["nc"], in_maps,
                               core_ids=list(range(NCORES)), trace=_trace)
    out = np.zeros((B, L, D), dtype=np.float32)
    for c in range(NCORES):
        out[c // 4] += res.results[c]["out"]
    if _trace:
        kernel._last_exec_time_ns = res.exec_time_ns
        kernel._last_profile = res
    return out
